# revision 19
# baseline (speedup 1.0000x reference)
"""Trainium2 Bass kernel for nn_Attention_85564338471023.

Multi-head self-attention (B=16, N=1024 tokens, C=512, 8 heads x d=64) with
qkv projection, softmax attention, output projection and residual.

Sharding: pure data-parallel over batch -- 2 batch elements per NeuronCore,
no collectives. Host pre-transposes x (channels-on-partitions) and reorders
w_qkv columns so heads come in pairs that share 128-partition tiles.

Device algorithm per batch element (all matmuls float32r, full PE rate):
  qkT[j',n]  = w_qk_re.T @ xT      (j' pair-grouped: [kA|kB]x4 then [qA|qB]x4)
  v[y,u]     = x @ w_v_re          (natural token-major layout, pair-grouped)
  per head (row-group paired, 2 heads concurrent in PE array):
    scT[y,x] = kT.T @ qT           (scores TRANSPOSED: keys on partitions)
    ex       = exp(scT / 8)        (ScalarE, scale fused; max-sub skipped --
                                    scores are ~N(0,1), |s|<8 in practice)
    res[d,x] = v_h.T @ ex          (col-group paired; accumulated over y-tiles)
    s[x]     = ones(64).T @ ex     (denominator; the 64 ones-columns broadcast
                                    s across the 64 partitions of its head)
    rt       = res * recip(s)      (DVE reciprocal_approx_fast + multiply)
  out[x,c]   = rt.T @ w_out + resid   (resid = x + b_out + b_v@w_out, host)
"""

from contextlib import ExitStack

import numpy as np

import concourse.bacc as bacc
import concourse.bass as bass
import concourse.tile as tile
from concourse import mybir
from concourse.bass_utils import run_bass_kernel_spmd  # noqa: F401 (fallback path)

N_CORES = 8
B, HH, WW, C = 16, 32, 32, 512
N = HH * WW            # 1024 tokens
NH, DH = 8, 64
SCALE = DH ** -0.5     # 0.125
BL = B // N_CORES      # 2 batch elements per core
P = 128
F32 = mybir.dt.float32
F32R = mybir.dt.float32r

# matmul compute dtype: float32r streams 1 row/cycle at N>=256 (4x faster
# than plain fp32 on the PE) at ~tf32-ish precision -- projections + scores.
# The A*V matmul uses a fused [v_h | ones] stationary in bf16 with M=127
# columns: exactly-128-column 2-byte stationaries engage the compiler's
# fast-weight-load path and 4-byte fused stationaries use a 2-pass weight
# load; both were observed to corrupt results on HW. M=127 avoids both.
MM_DT = F32R
AV_DT = F32R


def _mm(ap):
    return ap


def build_program(repeat: int = 1) -> bass.Bass:
    nc = bacc.Bacc("TRN2", target_bir_lowering=False, debug=False)

    xT_d = nc.dram_tensor("xT", [BL, C, N], MM_DT, kind="ExternalInput")
    rs_d = nc.dram_tensor("resid", [BL, N, C], F32, kind="ExternalInput")
    wqk_d = nc.dram_tensor("wqk", [C, 1024], MM_DT, kind="ExternalInput")
    bqk_d = nc.dram_tensor("bqk", [1024], F32, kind="ExternalInput")
    wv_d = nc.dram_tensor("wv", [C, 512], MM_DT, kind="ExternalInput")
    wout_d = nc.dram_tensor("wout", [C, 512], MM_DT, kind="ExternalInput")
    vones_d = nc.dram_tensor("vones", [P, 512], MM_DT, kind="ExternalInput")
    out_d = nc.dram_tensor("out", [BL, N, C], F32, kind="ExternalOutput")

    with tile.TileContext(nc) as tc, ExitStack() as ctx:
        consts = ctx.enter_context(tc.tile_pool(name="consts", bufs=1))
        wpool = ctx.enter_context(tc.tile_pool(name="w", bufs=1))
        xt_pool = ctx.enter_context(tc.tile_pool(name="xt", bufs=8))
        qk_pool = ctx.enter_context(tc.tile_pool(name="qk", bufs=8))
        v_pool = ctx.enter_context(tc.tile_pool(name="v", bufs=8))
        ex_pool = ctx.enter_context(tc.tile_pool(name="ex", bufs=4))
        rt_pool = ctx.enter_context(tc.tile_pool(name="rt", bufs=8))
        misc = ctx.enter_context(tc.tile_pool(name="misc", bufs=2))
        # PSUM: 8 banks total. psb: 2 x [128,1024] = 4 banks (qk-proj, scores)
        # pss: tag res x4 x [128,512] = 4 banks (v-proj, AV accum, out-proj)
        psb = ctx.enter_context(tc.tile_pool(name="psb", bufs=2, space="PSUM"))
        pss = ctx.enter_context(tc.tile_pool(name="pss", bufs=4, space="PSUM"))

        bqk_sb = consts.tile([P, 8], F32, tag="bqk")
        nc.sync.dma_start(out=bqk_sb[:], in_=bqk_d.ap().rearrange("(t p) -> p t", p=P))

        wqk_sb, wv_sb, wout_sb = [], [], []
        for kc in range(4):
            t = wpool.tile([P, 1024], MM_DT, tag=f"wqk{kc}")
            nc.sync.dma_start(out=t[:], in_=wqk_d.ap()[kc * P:(kc + 1) * P, :])
            wqk_sb.append(t)
        for kc in range(4):
            t = wpool.tile([P, 512], MM_DT, tag=f"wv{kc}")
            nc.sync.dma_start(out=t[:], in_=wv_d.ap()[kc * P:(kc + 1) * P, :])
            wv_sb.append(t)
            t = wpool.tile([P, 512], MM_DT, tag=f"wout{kc}")
            nc.sync.dma_start(out=t[:], in_=wout_d.ap()[kc * P:(kc + 1) * P, :])
            wout_sb.append(t)

        for b in [b for _ in range(repeat) for b in range(BL)]:
            # ---- load xT (channels on partitions) ----
            xt = []
            for kc in range(4):
                t = xt_pool.tile([P, N], MM_DT)
                nc.sync.dma_start(out=t[:], in_=xT_d.ap()[b, kc * P:(kc + 1) * P, :])
                xt.append(t)

            # ---- qk projection: qkT[j-tile] = w_qk_tile.T @ xT ----
            qk = []
            for jt in range(8):
                ps = psb.tile([P, N], F32, tag="big")
                for kc in range(4):
                    for xs in range(2):
                        nc.tensor.matmul(
                            ps[:, xs * 512:(xs + 1) * 512],
                            _mm(wqk_sb[kc][:, jt * P:(jt + 1) * P]),
                            _mm(xt[kc][:, xs * 512:(xs + 1) * 512]),
                            start=(kc == 0), stop=(kc == 3),
                        )
                t = qk_pool.tile([P, N], MM_DT)
                nc.vector.tensor_scalar(
                    out=t[:], in0=ps[:],
                    scalar1=bqk_sb[:, jt:jt + 1], scalar2=None,
                    op0=mybir.AluOpType.add,
                )
                qk.append(t)

            # ---- v projection (token-major) + ones-augmented stationaries ----
            # vo[yt] holds, per head h, the 128-col block [v_h | ones]: the
            # A*V matmul with this stationary computes res_h (rows 0-63) AND
            # the softmax denominator s_h broadcast 64-wide (rows 64-127) in
            # one pass over the exp stream.
            vo = []
            for yt in range(8):
                ps = pss.tile([P, 512], F32, tag="res")
                for kc in range(4):
                    nc.tensor.matmul(
                        ps[:],
                        _mm(xt[kc][:, yt * P:(yt + 1) * P]),
                        _mm(wv_sb[kc][:]),
                        start=(kc == 0), stop=(kc == 3),
                    )
                t = v_pool.tile([P, 1024], AV_DT)
                tv = t[:].rearrange("p (h c) -> p h c", h=8)
                nc.sync.dma_start(
                    out=tv[:, :, 64:128],
                    in_=vones_d.ap().rearrange("p (h c) -> p h c", h=8))
                nc.vector.tensor_copy(
                    tv[:, :, 0:64], ps[:].rearrange("p (h c) -> p h c", h=8))
                vo.append(t)

            # ---- attention, head pairs ----
            rt = {}
            for p in range(4):
                kk = qk[p]
                qq = qk[4 + p]
                ra_ps = {}
                for e in range(2):
                    for xs in range(2):
                        z = pss.tile([P, 512], F32, tag="res", name=f"ra_{b}_{p}_{e}_{xs}")
                        nc.vector.memset(z[:], 0.0)
                        ra_ps[(e, xs)] = z
                def emit_av(yt, ex_pair):
                    last = yt == 7
                    for e in range(2):
                        h = 2 * p + e
                        for xs in range(2):
                            nc.tensor.matmul(
                                ra_ps[(e, xs)][:],
                                vo[yt][:, h * 128:(h + 1) * 128],
                                ex_pair[e][:, xs * 512:(xs + 1) * 512],
                                start=False, stop=last, skip_group_check=True,
                            )

                prev = None
                for yt in range(8):
                    ex = []
                    for e in range(2):
                        sc = psb.tile([P, N], F32, tag="big")
                        for xs in range(2):
                            nc.tensor.matmul(
                                sc[:, xs * 512:(xs + 1) * 512],
                                _mm(kk[e * 64:(e + 1) * 64, yt * P:(yt + 1) * P]),
                                _mm(qq[e * 64:(e + 1) * 64, xs * 512:(xs + 1) * 512]),
                                start=True, stop=True,
                            )
                        t = ex_pool.tile([P, N], AV_DT)
                        nc.scalar.activation(
                            out=t[:], in_=sc[:],
                            func=mybir.ActivationFunctionType.Exp, scale=SCALE,
                        )
                        ex.append(t)
                    if prev is not None:
                        emit_av(*prev)
                    prev = (yt, ex)
                emit_av(*prev)
                # normalize. Each ra bank: rows 0-63 = res_h, rows 64-127 =
                # s_h broadcast. Assemble pair tiles with DMA partition
                # shifts; the custom-DVE reciprocal MUST run at partition
                # base 0 (it is a silent no-op at base 64 on HW).
                for xs in range(2):
                    ra = misc.tile([P, 512], F32, tag="prs")
                    nc.vector.tensor_copy(ra[:], ra_ps[(0, xs)][:])
                    rb = misc.tile([P, 512], F32, tag="rb")
                    nc.vector.tensor_copy(rb[:], ra_ps[(1, xs)][:])
                    pS = misc.tile([P, 512], F32, tag="pS")
                    nc.sync.dma_start(out=pS[0:64, :], in_=ra[64:128, :])
                    nc.vector.tensor_copy(pS[64:128, :], rb[64:128, :])
                    prec = misc.tile([P, 512], F32, tag="prec")
                    nc.vector.reciprocal_approx_fast(out=prec[:], in_=pS[:])
                    nc.sync.dma_start(out=ra[64:128, :], in_=rb[0:64, :])
                    t = rt_pool.tile([P, 512], MM_DT)
                    nc.vector.tensor_tensor(
                        out=t[:], in0=ra[:], in1=prec[:],
                        op=mybir.AluOpType.mult,
                    )
                    rt[(p, xs)] = t

            # ---- output projection + residual ----
            for nt in range(8):
                xs, sub = divmod(nt, 4)
                ps = pss.tile([P, 512], F32, tag="res")
                for p in range(4):
                    nc.tensor.matmul(
                        ps[:],
                        _mm(rt[(p, xs)][:, sub * P:(sub + 1) * P]),
                        _mm(wout_sb[p][:]),
                        start=(p == 0), stop=(p == 3),
                    )
                rs = misc.tile([P, 512], F32, tag="rs")
                nc.sync.dma_start(out=rs[:], in_=rs_d.ap()[b, nt * P:(nt + 1) * P, :])
                ob = misc.tile([P, 512], F32, tag="ob")
                nc.vector.tensor_tensor(
                    out=ob[:], in0=ps[:], in1=rs[:], op=mybir.AluOpType.add,
                )
                nc.sync.dma_start(out=out_d.ap()[b, nt * P:(nt + 1) * P, :], in_=ob[:])

    nc.compile()
    return nc


def host_prep(ft, w_qkv, b_qkv, w_out, b_out):
    ft = np.asarray(ft, dtype=np.float32)
    w_qkv = np.asarray(w_qkv, dtype=np.float32)
    b_qkv = np.asarray(b_qkv, dtype=np.float32)
    w_out = np.asarray(w_out, dtype=np.float32)
    b_out = np.asarray(b_out, dtype=np.float32)

    x = ft.reshape(B, N, C)
    xT = np.ascontiguousarray(x.transpose(0, 2, 1))

    w_qk_re = np.empty((C, 1024), np.float32)
    b_qk_re = np.empty((1024,), np.float32)
    w_v_re = np.empty((C, 512), np.float32)
    for p in range(4):
        hA, hB = 2 * p, 2 * p + 1
        w_qk_re[:, p * 128:p * 128 + 64] = w_qkv[:, hA * 192 + 64:hA * 192 + 128]
        w_qk_re[:, p * 128 + 64:p * 128 + 128] = w_qkv[:, hB * 192 + 64:hB * 192 + 128]
        b_qk_re[p * 128:p * 128 + 64] = b_qkv[hA * 192 + 64:hA * 192 + 128]
        b_qk_re[p * 128 + 64:p * 128 + 128] = b_qkv[hB * 192 + 64:hB * 192 + 128]
        w_qk_re[:, 512 + p * 128:512 + p * 128 + 64] = w_qkv[:, hA * 192:hA * 192 + 64]
        w_qk_re[:, 512 + p * 128 + 64:512 + p * 128 + 128] = w_qkv[:, hB * 192:hB * 192 + 64]
        b_qk_re[512 + p * 128:512 + p * 128 + 64] = b_qkv[hA * 192:hA * 192 + 64]
        b_qk_re[512 + p * 128 + 64:512 + p * 128 + 128] = b_qkv[hB * 192:hB * 192 + 64]
        w_v_re[:, p * 128:p * 128 + 64] = w_qkv[:, hA * 192 + 128:hA * 192 + 192]
        w_v_re[:, p * 128 + 64:p * 128 + 128] = w_qkv[:, hB * 192 + 128:hB * 192 + 192]

    b_v_nat = np.empty((512,), np.float32)
    for h in range(NH):
        b_v_nat[h * 64:(h + 1) * 64] = b_qkv[h * 192 + 128:h * 192 + 192]
    resid = x + b_out[None, None, :] + (b_v_nat @ w_out)[None, None, :]
    resid = np.ascontiguousarray(resid, dtype=np.float32)
    return xT, resid, w_qk_re, b_qk_re, w_v_re, np.ascontiguousarray(w_out)


_NC_CACHE = {}


def get_program(repeat: int = 1) -> bass.Bass:
    if repeat not in _NC_CACHE:
        _NC_CACHE[repeat] = build_program(repeat)
    return _NC_CACHE[repeat]


def make_in_maps(ft, w_qkv, b_qkv, w_out, b_out):
    xT, resid, w_qk_re, b_qk_re, w_v_re, w_out_c = host_prep(
        ft, w_qkv, b_qkv, w_out, b_out)
    in_maps = []
    for core in range(N_CORES):
        sl = slice(core * BL, (core + 1) * BL)
        in_maps.append({
            "xT": np.ascontiguousarray(xT[sl]),
            "resid": np.ascontiguousarray(resid[sl]),
            "wqk": w_qk_re,
            "bqk": b_qk_re,
            "wv": w_v_re,
            "wout": w_out_c,
            "vones": np.ones((P, 512), np.float32),
        })
    return in_maps


_RUNNER_CACHE = {}


def make_runner(repeat: int = 1):
    """Build (once) a persistent jitted executor for the bass program.

    Returns run(in_maps) -> list[dict[name, np.ndarray]] per core. Keeping
    the jitted callable alive means repeat calls skip lowering/compile and
    are pure dispatch+execute.
    """
    if repeat in _RUNNER_CACHE:
        return _RUNNER_CACHE[repeat]

    import jax
    from jax.experimental.shard_map import shard_map
    from jax.sharding import Mesh, PartitionSpec
    from concourse import mybir as _mb
    from concourse import bass2jax

    bass2jax.install_neuronx_cc_hook()
    nc = get_program(repeat)

    partition_name = nc.partition_id_tensor.name if nc.partition_id_tensor else None
    in_names, out_names, out_avals, zero_shapes = [], [], [], []
    for alloc in nc.m.functions[0].allocations:
        if not isinstance(alloc, _mb.MemoryLocationSet):
            continue
        name = alloc.memorylocations[0].name
        if alloc.kind == "ExternalInput":
            if name != partition_name:
                in_names.append(name)
        elif alloc.kind == "ExternalOutput":
            np_dt = _mb.dt.np(alloc.dtype)
            out_names.append(name)
            out_avals.append(jax.core.ShapedArray(tuple(alloc.tensor_shape), np_dt))
            zero_shapes.append((tuple(alloc.tensor_shape), np_dt))
    n_params = len(in_names)
    all_in_names = list(in_names) + list(out_names)
    if partition_name is not None:
        all_in_names.append(partition_name)

    def _body(*args):
        operands = list(args)
        if partition_name is not None:
            operands.append(bass2jax.partition_id_tensor())
        outs = bass2jax._bass_exec_p.bind(
            *operands,
            out_avals=tuple(out_avals),
            in_names=tuple(all_in_names),
            out_names=tuple(out_names),
            lowering_input_output_aliases=(),
            sim_require_finite=True,
            sim_require_nnan=True,
            nc=nc,
        )
        return tuple(outs)

    devices = jax.devices()[:N_CORES]
    mesh = Mesh(np.asarray(devices), ("core",))
    n_outs = len(out_names)
    sharded = jax.jit(
        shard_map(_body, mesh=mesh,
                  in_specs=(PartitionSpec("core"),) * (n_params + n_outs),
                  out_specs=(PartitionSpec("core"),) * n_outs,
                  check_rep=False),
        keep_unused=True,
    )

    def run(in_maps):
        concat_in = [
            np.concatenate([np.asarray(m[name]) for m in in_maps], axis=0)
            for name in in_names
        ]
        zeros = [np.zeros((N_CORES * s[0], *s[1:]), dt) for s, dt in zero_shapes]
        out_arrs = sharded(*concat_in, *zeros)
        return [
            {name: np.asarray(out_arrs[i]).reshape(N_CORES, *out_avals[i].shape)[c]
             for i, name in enumerate(out_names)}
            for c in range(N_CORES)
        ]

    def make_chained(k):
        def _chain(*args):
            ins = list(args[:n_params])
            bufs = list(args[n_params:])
            for _ in range(k):
                bufs = list(_body(*ins, *bufs))
            return tuple(bufs)
        return jax.jit(
            shard_map(_chain, mesh=mesh,
                      in_specs=(PartitionSpec("core"),) * (n_params + n_outs),
                      out_specs=(PartitionSpec("core"),) * n_outs,
                      check_rep=False),
            keep_unused=True,
        )

    run.sharded = sharded
    run.in_names = in_names
    run.zero_shapes = zero_shapes
    run.make_chained = make_chained
    run.mesh = mesh
    _RUNNER_CACHE[repeat] = run
    return run


def kernel(ft, w_qkv, b_qkv, w_out, b_out):
    run = make_runner()
    in_maps = make_in_maps(ft, w_qkv, b_qkv, w_out, b_out)
    results = run(in_maps)
    out = np.concatenate([r["out"] for r in results], axis=0)
    return out.reshape(B, HH, WW, C).astype(np.float32)


# revision 20
# speedup vs baseline: 1.0353x; 1.0353x over previous
"""Trainium2 Bass kernel for nn_Attention_85564338471023.

Multi-head self-attention (B=16, N=1024 tokens, C=512, 8 heads x d=64) with
qkv projection, softmax attention, output projection and residual.

Sharding: pure data-parallel over batch -- 2 batch elements per NeuronCore,
no collectives. Host pre-transposes x (channels-on-partitions) and reorders
w_qkv columns so heads come in pairs that share 128-partition tiles.

Device algorithm per batch element (all matmuls float32r, full PE rate):
  qkT[j',n]  = w_qk_re.T @ xT      (j' pair-grouped: [kA|kB]x4 then [qA|qB]x4)
  v[y,u]     = x @ w_v_re          (natural token-major layout, pair-grouped)
  per head (row-group paired, 2 heads concurrent in PE array):
    scT[y,x] = kT.T @ qT           (scores TRANSPOSED: keys on partitions)
    ex       = exp(scT / 8)        (ScalarE, scale fused; max-sub skipped --
                                    scores are ~N(0,1), |s|<8 in practice)
    res[d,x] = v_h.T @ ex          (col-group paired; accumulated over y-tiles)
    s[x]     = ones(64).T @ ex     (denominator; the 64 ones-columns broadcast
                                    s across the 64 partitions of its head)
    rt       = res * recip(s)      (DVE reciprocal_approx_fast + multiply)
  out[x,c]   = rt.T @ w_out + resid   (resid = x + b_out + b_v@w_out, host)
"""

from contextlib import ExitStack

import numpy as np

import concourse.bacc as bacc
import concourse.bass as bass
import concourse.tile as tile
from concourse import mybir
from concourse.bass_utils import run_bass_kernel_spmd  # noqa: F401 (fallback path)

N_CORES = 8
B, HH, WW, C = 16, 32, 32, 512
N = HH * WW            # 1024 tokens
NH, DH = 8, 64
SCALE = DH ** -0.5     # 0.125
BL = B // N_CORES      # 2 batch elements per core
P = 128
F32 = mybir.dt.float32
F32R = mybir.dt.float32r

# matmul compute dtype: float32r streams 1 row/cycle at N>=256 (4x faster
# than plain fp32 on the PE) at ~tf32-ish precision -- projections + scores.
# The A*V matmul uses a fused [v_h | ones] stationary in bf16 with M=127
# columns: exactly-128-column 2-byte stationaries engage the compiler's
# fast-weight-load path and 4-byte fused stationaries use a 2-pass weight
# load; both were observed to corrupt results on HW. M=127 avoids both.
MM_DT = F32R
AV_DT = F32R


def _mm(ap):
    return ap


def build_program(repeat: int = 1) -> bass.Bass:
    nc = bacc.Bacc("TRN2", target_bir_lowering=False, debug=False)

    xT_d = nc.dram_tensor("xT", [BL, C, N], MM_DT, kind="ExternalInput")
    rs_d = nc.dram_tensor("resid", [BL, N, C], F32, kind="ExternalInput")
    wqk_d = nc.dram_tensor("wqk", [C, 1024], MM_DT, kind="ExternalInput")
    bqk_d = nc.dram_tensor("bqk", [1024], F32, kind="ExternalInput")
    wv_d = nc.dram_tensor("wv", [C, 512], MM_DT, kind="ExternalInput")
    wout_d = nc.dram_tensor("wout", [C, 512], MM_DT, kind="ExternalInput")
    out_d = nc.dram_tensor("out", [BL, N, C], F32, kind="ExternalOutput")

    with tile.TileContext(nc) as tc, ExitStack() as ctx:
        consts = ctx.enter_context(tc.tile_pool(name="consts", bufs=1))
        wpool = ctx.enter_context(tc.tile_pool(name="w", bufs=1))
        xt_pool = ctx.enter_context(tc.tile_pool(name="xt", bufs=8))
        qk_pool = ctx.enter_context(tc.tile_pool(name="qk", bufs=8))
        v_pool = ctx.enter_context(tc.tile_pool(name="v", bufs=8))
        ex_pool = ctx.enter_context(tc.tile_pool(name="ex", bufs=4))
        rt_pool = ctx.enter_context(tc.tile_pool(name="rt", bufs=8))
        misc = ctx.enter_context(tc.tile_pool(name="misc", bufs=2))
        # PSUM: 8 banks total. psb: 2 x [128,1024] = 4 banks (qk-proj, scores)
        # pss: tag res x4 x [128,512] = 4 banks (v-proj, AV accum, out-proj)
        psb = ctx.enter_context(tc.tile_pool(name="psb", bufs=2, space="PSUM"))
        pss = ctx.enter_context(tc.tile_pool(name="pss", bufs=4, space="PSUM"))

        ones_f32 = consts.tile([P, 64], F32, tag="ones")
        nc.vector.memset(ones_f32[:], 1.0)
        bqk_sb = consts.tile([P, 8], F32, tag="bqk")
        nc.sync.dma_start(out=bqk_sb[:], in_=bqk_d.ap().rearrange("(t p) -> p t", p=P))

        wqk_sb, wv_sb, wout_sb = [], [], []
        for kc in range(4):
            t = wpool.tile([P, 1024], MM_DT, tag=f"wqk{kc}")
            nc.sync.dma_start(out=t[:], in_=wqk_d.ap()[kc * P:(kc + 1) * P, :])
            wqk_sb.append(t)
        for kc in range(4):
            t = wpool.tile([P, 512], MM_DT, tag=f"wv{kc}")
            nc.sync.dma_start(out=t[:], in_=wv_d.ap()[kc * P:(kc + 1) * P, :])
            wv_sb.append(t)
            t = wpool.tile([P, 512], MM_DT, tag=f"wout{kc}")
            nc.sync.dma_start(out=t[:], in_=wout_d.ap()[kc * P:(kc + 1) * P, :])
            wout_sb.append(t)

        for b in [b for _ in range(repeat) for b in range(BL)]:
            # ---- load xT (channels on partitions) ----
            xt = []
            for kc in range(4):
                t = xt_pool.tile([P, N], MM_DT)
                nc.sync.dma_start(out=t[:], in_=xT_d.ap()[b, kc * P:(kc + 1) * P, :])
                xt.append(t)

            # ---- qk projection: qkT[j-tile] = w_qk_tile.T @ xT ----
            qk = []
            for jt in range(8):
                ps = psb.tile([P, N], F32, tag="big")
                for kc in range(4):
                    for xs in range(2):
                        nc.tensor.matmul(
                            ps[:, xs * 512:(xs + 1) * 512],
                            _mm(wqk_sb[kc][:, jt * P:(jt + 1) * P]),
                            _mm(xt[kc][:, xs * 512:(xs + 1) * 512]),
                            start=(kc == 0), stop=(kc == 3),
                        )
                t = qk_pool.tile([P, N], MM_DT)
                nc.vector.tensor_scalar(
                    out=t[:], in0=ps[:],
                    scalar1=bqk_sb[:, jt:jt + 1], scalar2=None,
                    op0=mybir.AluOpType.add,
                )
                qk.append(t)

            # ---- v projection (token-major) + ones-augmented stationaries ----
            # vo[yt] holds, per head h, the 128-col block [v_h | ones]: the
            # A*V matmul with this stationary computes res_h (rows 0-63) AND
            # the softmax denominator s_h broadcast 64-wide (rows 64-127) in
            # one pass over the exp stream.
            vo = []
            for yt in range(8):
                ps = pss.tile([P, 512], F32, tag="res")
                for kc in range(4):
                    nc.tensor.matmul(
                        ps[:],
                        _mm(xt[kc][:, yt * P:(yt + 1) * P]),
                        _mm(wv_sb[kc][:]),
                        start=(kc == 0), stop=(kc == 3),
                    )
                t = v_pool.tile([P, 1024], AV_DT)
                tv = t[:].rearrange("p (h c) -> p h c", h=8)
                nc.vector.tensor_copy(
                    tv[:, :, 0:64], ps[:].rearrange("p (h c) -> p h c", h=8))
                nc.vector.tensor_copy(
                    tv[:, :, 64:128],
                    ones_f32[:, None, :].broadcast_to([P, 8, 64]))
                vo.append(t)

            # ---- attention, head pairs ----
            rt = {}
            for p in range(4):
                kk = qk[p]
                qq = qk[4 + p]
                ra_ps = {}
                for e in range(2):
                    for xs in range(2):
                        z = pss.tile([P, 512], F32, tag="res", name=f"ra_{b}_{p}_{e}_{xs}")
                        nc.vector.memset(z[:], 0.0)
                        ra_ps[(e, xs)] = z
                def emit_av(yt, ex_pair):
                    last = yt == 7
                    for e in range(2):
                        h = 2 * p + e
                        for xs in range(2):
                            nc.tensor.matmul(
                                ra_ps[(e, xs)][:],
                                vo[yt][:, h * 128:(h + 1) * 128],
                                ex_pair[e][:, xs * 512:(xs + 1) * 512],
                                start=False, stop=last, skip_group_check=True,
                            )

                prev = None
                for yt in range(8):
                    ex = []
                    for e in range(2):
                        sc = psb.tile([P, N], F32, tag="big")
                        for xs in range(2):
                            nc.tensor.matmul(
                                sc[:, xs * 512:(xs + 1) * 512],
                                _mm(kk[e * 64:(e + 1) * 64, yt * P:(yt + 1) * P]),
                                _mm(qq[e * 64:(e + 1) * 64, xs * 512:(xs + 1) * 512]),
                                start=True, stop=True,
                            )
                        t = ex_pool.tile([P, N], AV_DT)
                        nc.scalar.activation(
                            out=t[:], in_=sc[:],
                            func=mybir.ActivationFunctionType.Exp, scale=SCALE,
                        )
                        ex.append(t)
                    if prev is not None:
                        emit_av(*prev)
                    prev = (yt, ex)
                emit_av(*prev)
                # normalize. Each ra bank: rows 0-63 = res_h, rows 64-127 =
                # s_h broadcast. Assemble pair tiles with DMA partition
                # shifts; the custom-DVE reciprocal MUST run at partition
                # base 0 (it is a silent no-op at base 64 on HW).
                for xs in range(2):
                    ra = misc.tile([P, 512], F32, tag="prs")
                    nc.vector.tensor_copy(ra[:], ra_ps[(0, xs)][:])
                    rb = misc.tile([P, 512], F32, tag="rb")
                    nc.vector.tensor_copy(rb[:], ra_ps[(1, xs)][:])
                    pS = misc.tile([P, 512], F32, tag="pS")
                    nc.sync.dma_start(out=pS[0:64, :], in_=ra[64:128, :])
                    nc.vector.tensor_copy(pS[64:128, :], rb[64:128, :])
                    prec = misc.tile([P, 512], F32, tag="prec")
                    nc.vector.reciprocal_approx_fast(out=prec[:], in_=pS[:])
                    nc.sync.dma_start(out=ra[64:128, :], in_=rb[0:64, :])
                    t = rt_pool.tile([P, 512], MM_DT)
                    nc.vector.tensor_tensor(
                        out=t[:], in0=ra[:], in1=prec[:],
                        op=mybir.AluOpType.mult,
                    )
                    rt[(p, xs)] = t

            # ---- output projection + residual ----
            for nt in range(8):
                xs, sub = divmod(nt, 4)
                ps = pss.tile([P, 512], F32, tag="res")
                for p in range(4):
                    nc.tensor.matmul(
                        ps[:],
                        _mm(rt[(p, xs)][:, sub * P:(sub + 1) * P]),
                        _mm(wout_sb[p][:]),
                        start=(p == 0), stop=(p == 3),
                    )
                rs = misc.tile([P, 512], F32, tag="rs")
                nc.sync.dma_start(out=rs[:], in_=rs_d.ap()[b, nt * P:(nt + 1) * P, :])
                ob = misc.tile([P, 512], F32, tag="ob")
                nc.vector.tensor_tensor(
                    out=ob[:], in0=ps[:], in1=rs[:], op=mybir.AluOpType.add,
                )
                nc.sync.dma_start(out=out_d.ap()[b, nt * P:(nt + 1) * P, :], in_=ob[:])

    nc.compile()
    return nc


def host_prep(ft, w_qkv, b_qkv, w_out, b_out):
    ft = np.asarray(ft, dtype=np.float32)
    w_qkv = np.asarray(w_qkv, dtype=np.float32)
    b_qkv = np.asarray(b_qkv, dtype=np.float32)
    w_out = np.asarray(w_out, dtype=np.float32)
    b_out = np.asarray(b_out, dtype=np.float32)

    x = ft.reshape(B, N, C)
    xT = np.ascontiguousarray(x.transpose(0, 2, 1))

    w_qk_re = np.empty((C, 1024), np.float32)
    b_qk_re = np.empty((1024,), np.float32)
    w_v_re = np.empty((C, 512), np.float32)
    for p in range(4):
        hA, hB = 2 * p, 2 * p + 1
        w_qk_re[:, p * 128:p * 128 + 64] = w_qkv[:, hA * 192 + 64:hA * 192 + 128]
        w_qk_re[:, p * 128 + 64:p * 128 + 128] = w_qkv[:, hB * 192 + 64:hB * 192 + 128]
        b_qk_re[p * 128:p * 128 + 64] = b_qkv[hA * 192 + 64:hA * 192 + 128]
        b_qk_re[p * 128 + 64:p * 128 + 128] = b_qkv[hB * 192 + 64:hB * 192 + 128]
        w_qk_re[:, 512 + p * 128:512 + p * 128 + 64] = w_qkv[:, hA * 192:hA * 192 + 64]
        w_qk_re[:, 512 + p * 128 + 64:512 + p * 128 + 128] = w_qkv[:, hB * 192:hB * 192 + 64]
        b_qk_re[512 + p * 128:512 + p * 128 + 64] = b_qkv[hA * 192:hA * 192 + 64]
        b_qk_re[512 + p * 128 + 64:512 + p * 128 + 128] = b_qkv[hB * 192:hB * 192 + 64]
        w_v_re[:, p * 128:p * 128 + 64] = w_qkv[:, hA * 192 + 128:hA * 192 + 192]
        w_v_re[:, p * 128 + 64:p * 128 + 128] = w_qkv[:, hB * 192 + 128:hB * 192 + 192]

    b_v_nat = np.empty((512,), np.float32)
    for h in range(NH):
        b_v_nat[h * 64:(h + 1) * 64] = b_qkv[h * 192 + 128:h * 192 + 192]
    resid = x + b_out[None, None, :] + (b_v_nat @ w_out)[None, None, :]
    resid = np.ascontiguousarray(resid, dtype=np.float32)
    return xT, resid, w_qk_re, b_qk_re, w_v_re, np.ascontiguousarray(w_out)


_NC_CACHE = {}


def get_program(repeat: int = 1) -> bass.Bass:
    if repeat not in _NC_CACHE:
        _NC_CACHE[repeat] = build_program(repeat)
    return _NC_CACHE[repeat]


def make_in_maps(ft, w_qkv, b_qkv, w_out, b_out):
    xT, resid, w_qk_re, b_qk_re, w_v_re, w_out_c = host_prep(
        ft, w_qkv, b_qkv, w_out, b_out)
    in_maps = []
    for core in range(N_CORES):
        sl = slice(core * BL, (core + 1) * BL)
        in_maps.append({
            "xT": np.ascontiguousarray(xT[sl]),
            "resid": np.ascontiguousarray(resid[sl]),
            "wqk": w_qk_re,
            "bqk": b_qk_re,
            "wv": w_v_re,
            "wout": w_out_c,
        })
    return in_maps


_RUNNER_CACHE = {}


def make_runner(repeat: int = 1):
    """Build (once) a persistent jitted executor for the bass program.

    Returns run(in_maps) -> list[dict[name, np.ndarray]] per core. Keeping
    the jitted callable alive means repeat calls skip lowering/compile and
    are pure dispatch+execute.
    """
    if repeat in _RUNNER_CACHE:
        return _RUNNER_CACHE[repeat]

    import jax
    from jax.experimental.shard_map import shard_map
    from jax.sharding import Mesh, PartitionSpec
    from concourse import mybir as _mb
    from concourse import bass2jax

    bass2jax.install_neuronx_cc_hook()
    nc = get_program(repeat)

    partition_name = nc.partition_id_tensor.name if nc.partition_id_tensor else None
    in_names, out_names, out_avals, zero_shapes = [], [], [], []
    for alloc in nc.m.functions[0].allocations:
        if not isinstance(alloc, _mb.MemoryLocationSet):
            continue
        name = alloc.memorylocations[0].name
        if alloc.kind == "ExternalInput":
            if name != partition_name:
                in_names.append(name)
        elif alloc.kind == "ExternalOutput":
            np_dt = _mb.dt.np(alloc.dtype)
            out_names.append(name)
            out_avals.append(jax.core.ShapedArray(tuple(alloc.tensor_shape), np_dt))
            zero_shapes.append((tuple(alloc.tensor_shape), np_dt))
    n_params = len(in_names)
    all_in_names = list(in_names) + list(out_names)
    if partition_name is not None:
        all_in_names.append(partition_name)

    def _body(*args):
        operands = list(args)
        if partition_name is not None:
            operands.append(bass2jax.partition_id_tensor())
        outs = bass2jax._bass_exec_p.bind(
            *operands,
            out_avals=tuple(out_avals),
            in_names=tuple(all_in_names),
            out_names=tuple(out_names),
            lowering_input_output_aliases=(),
            sim_require_finite=True,
            sim_require_nnan=True,
            nc=nc,
        )
        return tuple(outs)

    devices = jax.devices()[:N_CORES]
    mesh = Mesh(np.asarray(devices), ("core",))
    n_outs = len(out_names)
    sharded = jax.jit(
        shard_map(_body, mesh=mesh,
                  in_specs=(PartitionSpec("core"),) * (n_params + n_outs),
                  out_specs=(PartitionSpec("core"),) * n_outs,
                  check_rep=False),
        keep_unused=True,
    )

    def run(in_maps):
        concat_in = [
            np.concatenate([np.asarray(m[name]) for m in in_maps], axis=0)
            for name in in_names
        ]
        zeros = [np.zeros((N_CORES * s[0], *s[1:]), dt) for s, dt in zero_shapes]
        out_arrs = sharded(*concat_in, *zeros)
        return [
            {name: np.asarray(out_arrs[i]).reshape(N_CORES, *out_avals[i].shape)[c]
             for i, name in enumerate(out_names)}
            for c in range(N_CORES)
        ]

    def make_chained(k):
        def _chain(*args):
            ins = list(args[:n_params])
            bufs = list(args[n_params:])
            for _ in range(k):
                bufs = list(_body(*ins, *bufs))
            return tuple(bufs)
        return jax.jit(
            shard_map(_chain, mesh=mesh,
                      in_specs=(PartitionSpec("core"),) * (n_params + n_outs),
                      out_specs=(PartitionSpec("core"),) * n_outs,
                      check_rep=False),
            keep_unused=True,
        )

    run.sharded = sharded
    run.in_names = in_names
    run.zero_shapes = zero_shapes
    run.make_chained = make_chained
    run.mesh = mesh
    _RUNNER_CACHE[repeat] = run
    return run


def kernel(ft, w_qkv, b_qkv, w_out, b_out):
    run = make_runner()
    in_maps = make_in_maps(ft, w_qkv, b_qkv, w_out, b_out)
    results = run(in_maps)
    out = np.concatenate([r["out"] for r in results], axis=0)
    return out.reshape(B, HH, WW, C).astype(np.float32)


# revision 21
# speedup vs baseline: 1.5540x; 1.5010x over previous
"""Trainium2 Bass kernel for nn_Attention_85564338471023.

Multi-head self-attention (B=16, N=1024 tokens, C=512, 8 heads x d=64) with
qkv projection, softmax attention, output projection and residual.

Sharding: pure data-parallel over batch -- 2 batch elements per NeuronCore,
no collectives. Host pre-transposes x (channels-on-partitions) and reorders
w_qkv columns so heads come in pairs that share 128-partition tiles.

Device algorithm per batch element (all matmuls float32r, full PE rate):
  qkT[j',n]  = w_qk_re.T @ xT      (j' pair-grouped: [kA|kB]x4 then [qA|qB]x4)
  v[y,u]     = x @ w_v_re          (natural token-major layout, pair-grouped)
  per head (row-group paired, 2 heads concurrent in PE array):
    scT[y,x] = kT.T @ qT           (scores TRANSPOSED: keys on partitions)
    ex       = exp(scT / 8)        (ScalarE, scale fused; max-sub skipped --
                                    scores are ~N(0,1), |s|<8 in practice)
    res[d,x] = v_h.T @ ex          (col-group paired; accumulated over y-tiles)
    s[x]     = ones(64).T @ ex     (denominator; the 64 ones-columns broadcast
                                    s across the 64 partitions of its head)
    rt       = res * recip(s)      (DVE reciprocal_approx_fast + multiply)
  out[x,c]   = rt.T @ w_out + resid   (resid = x + b_out + b_v@w_out, host)
"""

from contextlib import ExitStack

import numpy as np

import concourse.bacc as bacc
import concourse.bass as bass
import concourse.tile as tile
from concourse import mybir
from concourse.bass_utils import run_bass_kernel_spmd  # noqa: F401 (fallback path)

N_CORES = 8
B, HH, WW, C = 16, 32, 32, 512
N = HH * WW            # 1024 tokens
NH, DH = 8, 64
SCALE = DH ** -0.5     # 0.125
BL = B // N_CORES      # 2 batch elements per core
P = 128
F32 = mybir.dt.float32
F32R = mybir.dt.float32r

# matmul compute dtype: float32r streams 1 row/cycle at N>=256 (4x faster
# than plain fp32 on the PE) at ~tf32-ish precision -- projections + scores.
# The A*V matmul uses a fused [v_h | ones] stationary in bf16 with M=127
# columns: exactly-128-column 2-byte stationaries engage the compiler's
# fast-weight-load path and 4-byte fused stationaries use a 2-pass weight
# load; both were observed to corrupt results on HW. M=127 avoids both.
MM_DT = F32R
AV_DT = mybir.dt.bfloat16


def _mm(ap):
    return ap


def build_program(repeat: int = 1) -> bass.Bass:
    nc = bacc.Bacc("TRN2", target_bir_lowering=False, debug=False)

    xT_d = nc.dram_tensor("xT", [BL, C, N], MM_DT, kind="ExternalInput")
    rs_d = nc.dram_tensor("resid", [BL, N, C], F32, kind="ExternalInput")
    wqk_d = nc.dram_tensor("wqk", [C, 1024], MM_DT, kind="ExternalInput")
    bqk_d = nc.dram_tensor("bqk", [1024], F32, kind="ExternalInput")
    wv_d = nc.dram_tensor("wv", [C, 512], MM_DT, kind="ExternalInput")
    wout_d = nc.dram_tensor("wout", [C, 512], MM_DT, kind="ExternalInput")
    out_d = nc.dram_tensor("out", [BL, N, C], F32, kind="ExternalOutput")

    with tile.TileContext(nc) as tc, ExitStack() as ctx:
        consts = ctx.enter_context(tc.tile_pool(name="consts", bufs=1))
        wpool = ctx.enter_context(tc.tile_pool(name="w", bufs=1))
        xt_pool = ctx.enter_context(tc.tile_pool(name="xt", bufs=8))
        qk_pool = ctx.enter_context(tc.tile_pool(name="qk", bufs=8))
        v_pool = ctx.enter_context(tc.tile_pool(name="v", bufs=8))
        ex_pool = ctx.enter_context(tc.tile_pool(name="ex", bufs=4))
        rt_pool = ctx.enter_context(tc.tile_pool(name="rt", bufs=8))
        misc = ctx.enter_context(tc.tile_pool(name="misc", bufs=2))
        # PSUM: 8 banks total. psb: 2 x [128,1024] = 4 banks (qk-proj, scores)
        # pss: tag res x4 x [128,512] = 4 banks (v-proj, AV accum, out-proj)
        psb = ctx.enter_context(tc.tile_pool(name="psb", bufs=2, space="PSUM"))
        pss = ctx.enter_context(tc.tile_pool(name="pss", bufs=4, space="PSUM"))

        ones = consts.tile([P, 64], AV_DT, tag="ones")
        nc.vector.memset(ones[:], 1.0)
        bqk_sb = consts.tile([P, 8], F32, tag="bqk")
        nc.sync.dma_start(out=bqk_sb[:], in_=bqk_d.ap().rearrange("(t p) -> p t", p=P))

        wqk_sb, wv_sb, wout_sb = [], [], []
        for kc in range(4):
            t = wpool.tile([P, 1024], MM_DT, tag=f"wqk{kc}")
            nc.sync.dma_start(out=t[:], in_=wqk_d.ap()[kc * P:(kc + 1) * P, :])
            wqk_sb.append(t)
        for kc in range(4):
            t = wpool.tile([P, 512], MM_DT, tag=f"wv{kc}")
            nc.sync.dma_start(out=t[:], in_=wv_d.ap()[kc * P:(kc + 1) * P, :])
            wv_sb.append(t)
            t = wpool.tile([P, 512], MM_DT, tag=f"wout{kc}")
            nc.sync.dma_start(out=t[:], in_=wout_d.ap()[kc * P:(kc + 1) * P, :])
            wout_sb.append(t)

        for b in [b for _ in range(repeat) for b in range(BL)]:
            # ---- load xT (channels on partitions) ----
            xt = []
            for kc in range(4):
                t = xt_pool.tile([P, N], MM_DT)
                nc.sync.dma_start(out=t[:], in_=xT_d.ap()[b, kc * P:(kc + 1) * P, :])
                xt.append(t)

            # ---- qk projection: qkT[j-tile] = w_qk_tile.T @ xT ----
            qk = []
            for jt in range(8):
                ps = psb.tile([P, N], F32, tag="big")
                for kc in range(4):
                    for xs in range(2):
                        nc.tensor.matmul(
                            ps[:, xs * 512:(xs + 1) * 512],
                            _mm(wqk_sb[kc][:, jt * P:(jt + 1) * P]),
                            _mm(xt[kc][:, xs * 512:(xs + 1) * 512]),
                            start=(kc == 0), stop=(kc == 3),
                        )
                t = qk_pool.tile([P, N], MM_DT)
                nc.vector.tensor_scalar(
                    out=t[:], in0=ps[:],
                    scalar1=bqk_sb[:, jt:jt + 1], scalar2=None,
                    op0=mybir.AluOpType.add,
                )
                qk.append(t)

            # ---- v projection (token-major) + ones-augmented stationaries ----
            # vo[yt] holds, per head h, the 128-col block [v_h | ones]: the
            # A*V matmul with this stationary computes res_h (rows 0-63) AND
            # the softmax denominator s_h broadcast 64-wide (rows 64-127) in
            # one pass over the exp stream.
            vo = []
            for yt in range(8):
                ps = pss.tile([P, 512], F32, tag="res")
                for kc in range(4):
                    nc.tensor.matmul(
                        ps[:],
                        _mm(xt[kc][:, yt * P:(yt + 1) * P]),
                        _mm(wv_sb[kc][:]),
                        start=(kc == 0), stop=(kc == 3),
                    )
                t = v_pool.tile([P, 512], AV_DT)
                nc.vector.tensor_copy(t[:], ps[:])
                vo.append(t)

            # ---- attention, head pairs ----
            rt = {}
            for p in range(4):
                kk = qk[p]
                qq = qk[4 + p]
                res_ps = [pss.tile([P, 512], F32, tag="res", name=f"res_{b}_{p}_{i}") for i in range(2)]
                s_ps = [pss.tile([P, 512], F32, tag="res", name=f"s_{b}_{p}_{i}") for i in range(2)]
                for z in (*res_ps, *s_ps):
                    nc.vector.memset(z[:], 0.0)
                def emit_av(yt, ex_pair):
                    last = yt == 7
                    for e in range(2):
                        h = 2 * p + e
                        for xs in range(2):
                            exs = ex_pair[e][:, xs * 512:(xs + 1) * 512]
                            nc.tensor.matmul(
                                res_ps[xs][e * 64:(e + 1) * 64, :],
                                vo[yt][:, h * 64:(h + 1) * 64],
                                exs,
                                start=False, stop=last, skip_group_check=True,
                            )
                            nc.tensor.matmul(
                                s_ps[xs][e * 64:(e + 1) * 64, :],
                                ones[:],
                                exs,
                                start=False, stop=last, skip_group_check=True,
                            )

                prev = None
                for yt in range(8):
                    ex = []
                    for e in range(2):
                        sc = psb.tile([P, N], F32, tag="big")
                        for xs in range(2):
                            nc.tensor.matmul(
                                sc[:, xs * 512:(xs + 1) * 512],
                                _mm(kk[e * 64:(e + 1) * 64, yt * P:(yt + 1) * P]),
                                _mm(qq[e * 64:(e + 1) * 64, xs * 512:(xs + 1) * 512]),
                                start=True, stop=True,
                            )
                        t = ex_pool.tile([P, N], AV_DT)
                        nc.scalar.activation(
                            out=t[:], in_=sc[:],
                            func=mybir.ActivationFunctionType.Exp, scale=SCALE,
                        )
                        ex.append(t)
                    if prev is not None:
                        emit_av(*prev)
                    prev = (yt, ex)
                emit_av(*prev)
                for xs in range(2):
                    rec = misc.tile([P, 512], F32, tag="prc")
                    nc.vector.reciprocal_approx_fast(out=rec[:], in_=s_ps[xs][:])
                    t = rt_pool.tile([P, 512], MM_DT)
                    nc.vector.tensor_tensor(
                        out=t[:], in0=res_ps[xs][:], in1=rec[:],
                        op=mybir.AluOpType.mult,
                    )
                    rt[(p, xs)] = t

            # ---- output projection + residual ----
            for nt in range(8):
                xs, sub = divmod(nt, 4)
                ps = pss.tile([P, 512], F32, tag="res")
                for p in range(4):
                    nc.tensor.matmul(
                        ps[:],
                        _mm(rt[(p, xs)][:, sub * P:(sub + 1) * P]),
                        _mm(wout_sb[p][:]),
                        start=(p == 0), stop=(p == 3),
                    )
                rs = misc.tile([P, 512], F32, tag="rs")
                nc.sync.dma_start(out=rs[:], in_=rs_d.ap()[b, nt * P:(nt + 1) * P, :])
                ob = misc.tile([P, 512], F32, tag="ob")
                nc.vector.tensor_tensor(
                    out=ob[:], in0=ps[:], in1=rs[:], op=mybir.AluOpType.add,
                )
                nc.sync.dma_start(out=out_d.ap()[b, nt * P:(nt + 1) * P, :], in_=ob[:])

    nc.compile()
    return nc


def host_prep(ft, w_qkv, b_qkv, w_out, b_out):
    ft = np.asarray(ft, dtype=np.float32)
    w_qkv = np.asarray(w_qkv, dtype=np.float32)
    b_qkv = np.asarray(b_qkv, dtype=np.float32)
    w_out = np.asarray(w_out, dtype=np.float32)
    b_out = np.asarray(b_out, dtype=np.float32)

    x = ft.reshape(B, N, C)
    xT = np.ascontiguousarray(x.transpose(0, 2, 1))

    w_qk_re = np.empty((C, 1024), np.float32)
    b_qk_re = np.empty((1024,), np.float32)
    w_v_re = np.empty((C, 512), np.float32)
    for p in range(4):
        hA, hB = 2 * p, 2 * p + 1
        w_qk_re[:, p * 128:p * 128 + 64] = w_qkv[:, hA * 192 + 64:hA * 192 + 128]
        w_qk_re[:, p * 128 + 64:p * 128 + 128] = w_qkv[:, hB * 192 + 64:hB * 192 + 128]
        b_qk_re[p * 128:p * 128 + 64] = b_qkv[hA * 192 + 64:hA * 192 + 128]
        b_qk_re[p * 128 + 64:p * 128 + 128] = b_qkv[hB * 192 + 64:hB * 192 + 128]
        w_qk_re[:, 512 + p * 128:512 + p * 128 + 64] = w_qkv[:, hA * 192:hA * 192 + 64]
        w_qk_re[:, 512 + p * 128 + 64:512 + p * 128 + 128] = w_qkv[:, hB * 192:hB * 192 + 64]
        b_qk_re[512 + p * 128:512 + p * 128 + 64] = b_qkv[hA * 192:hA * 192 + 64]
        b_qk_re[512 + p * 128 + 64:512 + p * 128 + 128] = b_qkv[hB * 192:hB * 192 + 64]
        w_v_re[:, p * 128:p * 128 + 64] = w_qkv[:, hA * 192 + 128:hA * 192 + 192]
        w_v_re[:, p * 128 + 64:p * 128 + 128] = w_qkv[:, hB * 192 + 128:hB * 192 + 192]

    b_v_nat = np.empty((512,), np.float32)
    for h in range(NH):
        b_v_nat[h * 64:(h + 1) * 64] = b_qkv[h * 192 + 128:h * 192 + 192]
    resid = x + b_out[None, None, :] + (b_v_nat @ w_out)[None, None, :]
    resid = np.ascontiguousarray(resid, dtype=np.float32)
    return xT, resid, w_qk_re, b_qk_re, w_v_re, np.ascontiguousarray(w_out)


_NC_CACHE = {}


def get_program(repeat: int = 1) -> bass.Bass:
    if repeat not in _NC_CACHE:
        _NC_CACHE[repeat] = build_program(repeat)
    return _NC_CACHE[repeat]


def make_in_maps(ft, w_qkv, b_qkv, w_out, b_out):
    xT, resid, w_qk_re, b_qk_re, w_v_re, w_out_c = host_prep(
        ft, w_qkv, b_qkv, w_out, b_out)
    in_maps = []
    for core in range(N_CORES):
        sl = slice(core * BL, (core + 1) * BL)
        in_maps.append({
            "xT": np.ascontiguousarray(xT[sl]),
            "resid": np.ascontiguousarray(resid[sl]),
            "wqk": w_qk_re,
            "bqk": b_qk_re,
            "wv": w_v_re,
            "wout": w_out_c,
        })
    return in_maps


_RUNNER_CACHE = {}


def make_runner(repeat: int = 1):
    """Build (once) a persistent jitted executor for the bass program.

    Returns run(in_maps) -> list[dict[name, np.ndarray]] per core. Keeping
    the jitted callable alive means repeat calls skip lowering/compile and
    are pure dispatch+execute.
    """
    if repeat in _RUNNER_CACHE:
        return _RUNNER_CACHE[repeat]

    import jax
    from jax.experimental.shard_map import shard_map
    from jax.sharding import Mesh, PartitionSpec
    from concourse import mybir as _mb
    from concourse import bass2jax

    bass2jax.install_neuronx_cc_hook()
    nc = get_program(repeat)

    partition_name = nc.partition_id_tensor.name if nc.partition_id_tensor else None
    in_names, out_names, out_avals, zero_shapes = [], [], [], []
    for alloc in nc.m.functions[0].allocations:
        if not isinstance(alloc, _mb.MemoryLocationSet):
            continue
        name = alloc.memorylocations[0].name
        if alloc.kind == "ExternalInput":
            if name != partition_name:
                in_names.append(name)
        elif alloc.kind == "ExternalOutput":
            np_dt = _mb.dt.np(alloc.dtype)
            out_names.append(name)
            out_avals.append(jax.core.ShapedArray(tuple(alloc.tensor_shape), np_dt))
            zero_shapes.append((tuple(alloc.tensor_shape), np_dt))
    n_params = len(in_names)
    all_in_names = list(in_names) + list(out_names)
    if partition_name is not None:
        all_in_names.append(partition_name)

    def _body(*args):
        operands = list(args)
        if partition_name is not None:
            operands.append(bass2jax.partition_id_tensor())
        outs = bass2jax._bass_exec_p.bind(
            *operands,
            out_avals=tuple(out_avals),
            in_names=tuple(all_in_names),
            out_names=tuple(out_names),
            lowering_input_output_aliases=(),
            sim_require_finite=True,
            sim_require_nnan=True,
            nc=nc,
        )
        return tuple(outs)

    devices = jax.devices()[:N_CORES]
    mesh = Mesh(np.asarray(devices), ("core",))
    n_outs = len(out_names)
    sharded = jax.jit(
        shard_map(_body, mesh=mesh,
                  in_specs=(PartitionSpec("core"),) * (n_params + n_outs),
                  out_specs=(PartitionSpec("core"),) * n_outs,
                  check_rep=False),
        keep_unused=True,
    )

    def run(in_maps):
        concat_in = [
            np.concatenate([np.asarray(m[name]) for m in in_maps], axis=0)
            for name in in_names
        ]
        zeros = [np.zeros((N_CORES * s[0], *s[1:]), dt) for s, dt in zero_shapes]
        out_arrs = sharded(*concat_in, *zeros)
        return [
            {name: np.asarray(out_arrs[i]).reshape(N_CORES, *out_avals[i].shape)[c]
             for i, name in enumerate(out_names)}
            for c in range(N_CORES)
        ]

    def make_chained(k):
        def _chain(*args):
            ins = list(args[:n_params])
            bufs = list(args[n_params:])
            for _ in range(k):
                bufs = list(_body(*ins, *bufs))
            return tuple(bufs)
        return jax.jit(
            shard_map(_chain, mesh=mesh,
                      in_specs=(PartitionSpec("core"),) * (n_params + n_outs),
                      out_specs=(PartitionSpec("core"),) * n_outs,
                      check_rep=False),
            keep_unused=True,
        )

    run.sharded = sharded
    run.in_names = in_names
    run.zero_shapes = zero_shapes
    run.make_chained = make_chained
    run.mesh = mesh
    _RUNNER_CACHE[repeat] = run
    return run


def kernel(ft, w_qkv, b_qkv, w_out, b_out):
    run = make_runner()
    in_maps = make_in_maps(ft, w_qkv, b_qkv, w_out, b_out)
    results = run(in_maps)
    out = np.concatenate([r["out"] for r in results], axis=0)
    return out.reshape(B, HH, WW, C).astype(np.float32)
